# revision 5
# baseline (speedup 1.0000x reference)
"""Trainium2 Bass kernel for 3D self-attention (nn_Attention3D).

Reference computation per batch b (C=128 channels, N=W*L*H=4096 tokens, d=16):
    xf = x[b].reshape(C, N)
    q = wq @ xf            # [d, N]
    k = wk @ xf            # [d, N]
    v = wv @ xf            # [C, N]
    E = q.T @ k            # [N, N]
    A = softmax(E, axis=-1)
    out = gamma * (v @ A.T) + xf   # [C, N]
    returns (out.reshape(B,C,W,L,H), A)

Sharding: data-parallel over batch B=8 across the 8 NeuronCores (one batch
per core); the tiny projection weights are replicated.

Per-core kernel strategy:
  - All matmuls in bf16 (fp32 PSUM accumulation).
  - softmax without max subtraction (E values are small; exp is safe in
    fp32/bf16 range): P = exp(E), s = rowsum(P) fused into the activation,
    A = P * (1/s).
  - A is produced in [i, j] layout (softmax rows on partitions); the AV
    matmul needs P^T ([j, i] layout), produced by PE transposes of the bf16
    P tiles (bf16 PSUM output) + batched DVE copies.
  - out[c, i] = sum_j v[c, j] P[i, j] accumulated in PSUM over j-tiles with
    vT[j, c] stationary; normalized by gamma/s[i] broadcast across partitions
    via a tiny ones-matmul, then residual-added with x.
"""

import numpy as np
import ml_dtypes
from contextlib import ExitStack

B, C, N, D = 8, 128, 4096, 16
W = L = H = 16
NT = N // 128   # 32 tiles of 128
NS = N // 512   # 8 superblocks of 512

_cache = {}


def _build():
    import concourse.bass as bass
    import concourse.tile as tile
    from concourse import bacc, mybir
    from concourse.bass import ts
    from concourse.masks import make_identity

    f32 = mybir.dt.float32
    f32r = mybir.dt.float32r
    bf16 = mybir.dt.bfloat16
    Exp = mybir.ActivationFunctionType.Exp
    mult = mybir.AluOpType.mult
    add = mybir.AluOpType.add
    X = mybir.AxisListType.X

    nc = bacc.Bacc("TRN2", target_bir_lowering=False, num_devices=8)

    x_in = nc.dram_tensor("x", [C, N], f32, kind="ExternalInput")
    xb16_in = nc.dram_tensor("xb16", [C, N], bf16, kind="ExternalInput")
    wqt_in = nc.dram_tensor("wqt", [C, D], f32, kind="ExternalInput")
    wkt_in = nc.dram_tensor("wkt", [C, D], f32, kind="ExternalInput")
    wvt_in = nc.dram_tensor("wvt", [C, C], bf16, kind="ExternalInput")
    gam_in = nc.dram_tensor("gam", [128, 1], f32, kind="ExternalInput")
    attn_out = nc.dram_tensor("attn", [N, N], f32, kind="ExternalOutput")
    out_out = nc.dram_tensor("out", [C, N], f32, kind="ExternalOutput")

    with tile.TileContext(nc) as tc, ExitStack() as ctx:
        consts = ctx.enter_context(tc.tile_pool(name="consts", bufs=1))
        epool = ctx.enter_context(tc.tile_pool(name="epsum", bufs=2, space="PSUM"))
        tpool = ctx.enter_context(tc.tile_pool(name="tpsum", bufs=2, space="PSUM"))
        apool = ctx.enter_context(tc.tile_pool(name="avpsum", bufs=1, space="PSUM"))
        spool = ctx.enter_context(tc.tile_pool(name="smpsum", bufs=1, space="PSUM"))
        work = ctx.enter_context(tc.tile_pool(name="work", bufs=2))
        ptp = ctx.enter_context(tc.tile_pool(name="pts", bufs=1))

        # ---- constants / inputs ----
        wqt = consts.tile([C, D], f32)
        nc.sync.dma_start(wqt[:], wqt_in[:, :])
        wkt = consts.tile([C, D], f32)
        nc.sync.dma_start(wkt[:], wkt_in[:, :])
        wvt = consts.tile([C, C], bf16)
        nc.sync.dma_start(wvt[:], wvt_in[:, :])
        gam = consts.tile([128, 1], f32)
        nc.sync.dma_start(gam[:], gam_in[:, :])
        xb = consts.tile([C, N], f32)
        nc.sync.dma_start(xb[:], x_in[:, :])
        xb16 = consts.tile([C, N], bf16)
        nc.sync.dma_start(xb16[:], xb16_in[:, :])
        x32r = consts.tile([C, N], f32r)
        nc.vector.tensor_copy(x32r[:], xb[:])
        wq32r = consts.tile([C, D], f32r)
        nc.vector.tensor_copy(wq32r[:], wqt[:])
        wk32r = consts.tile([C, D], f32r)
        nc.vector.tensor_copy(wk32r[:], wkt[:])

        ident16 = consts.tile([128, 128], bf16)
        make_identity(nc, ident16[:])
        ident32 = consts.tile([128, 128], f32)
        make_identity(nc, ident32[:])
        ones1 = consts.tile([1, 128], f32)
        nc.vector.memset(ones1[:], 1.0)

        qsb = consts.tile([D, N], f32r)
        ksb = consts.tile([D, N], f32r)
        vsb = consts.tile([C, N], bf16)
        vT = consts.tile([128, N], bf16)  # [j within tile, jt*128 + c]

        # ---- projections ----
        for blk in range(NS):
            qp = epool.tile([D, 512], f32, tag="ep")
            nc.tensor.matmul(qp[:], wq32r[:], x32r[:, ts(blk, 512)], start=True, stop=True)
            nc.vector.tensor_copy(qsb[:, ts(blk, 512)], qp[:])
            kp = epool.tile([D, 512], f32, tag="ep")
            nc.tensor.matmul(kp[:], wk32r[:], x32r[:, ts(blk, 512)], start=True, stop=True)
            nc.vector.tensor_copy(ksb[:, ts(blk, 512)], kp[:])
            vp = epool.tile([C, 512], f32, tag="ep")
            nc.tensor.matmul(vp[:], wvt[:], xb16[:, ts(blk, 512)], start=True, stop=True)
            nc.vector.tensor_copy(vsb[:, ts(blk, 512)], vp[:])

        # ---- v transpose: vT[:, jt*128 + c] = v[c, jt*128 + j] ----
        for tb in range(4):
            tp = tpool.tile([128, 1024], bf16, tag="tps")
            for u in range(8):
                jt = tb * 8 + u
                nc.tensor.transpose(tp[:, ts(u, 128)], vsb[:, ts(jt, 128)], ident16[:])
            nc.vector.tensor_copy(vT[:, ts(tb, 1024)], tp[:])

        # ---- main loop over i-superblocks ----
        for S in range(NS):
            PTs = ptp.tile([128, NT * 512], bf16, tag="pt")
            PTs_v = PTs[:].rearrange("p (jt i) -> p jt i", jt=NT)
            rTs = work.tile([1, 512], f32, tag="rts")
            for Iti in range(4):
                it = S * 4 + Iti
                Pb = work.tile([128, N], bf16, tag="pb")
                spart = work.tile([128, 4], f32, tag="sp")
                for eb in range(4):
                    ep = epool.tile([128, 1024], f32, tag="ep")
                    nc.tensor.matmul(ep[:, 0:512], qsb[:, ts(it, 128)],
                                     ksb[:, ts(2 * eb, 512)], start=True, stop=True)
                    nc.tensor.matmul(ep[:, 512:1024], qsb[:, ts(it, 128)],
                                     ksb[:, ts(2 * eb + 1, 512)], start=True, stop=True)
                    nc.scalar.activation(Pb[:, ts(eb, 1024)], ep[:], Exp,
                                         accum_out=spart[:, eb:eb + 1])
                s = work.tile([128, 1], f32, tag="s")
                nc.vector.tensor_reduce(s[:], spart[:], axis=X, op=add)
                r = work.tile([128, 1], f32, tag="r")
                nc.vector.reciprocal(r[:], s[:])

                # A = P * (1/s)  -> fp32 rows to HBM
                Asb = work.tile([128, N], f32, tag="a")
                nc.vector.tensor_scalar_mul(Asb[:], Pb[:], r[:])
                nc.sync.dma_start(attn_out[ts(it, 128), :], Asb[:])

                # transposes of Pb into PTs
                for tb in range(4):
                    tp = tpool.tile([128, 1024], bf16, tag="tps")
                    for u in range(8):
                        jt = tb * 8 + u
                        nc.tensor.transpose(tp[:, ts(u, 128)], Pb[:, ts(jt, 128)],
                                            ident16[:])
                    tp_v = tp[:].rearrange("p (u i) -> p u i", u=8)
                    nc.vector.tensor_copy(
                        PTs_v[:, tb * 8:(tb + 1) * 8, Iti * 128:(Iti + 1) * 128], tp_v)

                # rT for the later broadcast: rTs[0, Iti*128 + i] = r[i]
                rt = spool.tile([1, 128], f32, tag="sm")
                nc.tensor.transpose(rt[:], r[:], ident32[:])
                nc.vector.tensor_copy(rTs[:, ts(Iti, 128)], rt[:])

            # ---- AV for this superblock ----
            acc = apool.tile([128, 512], f32, tag="av")
            for jt in range(NT):
                nc.tensor.matmul(acc[:], vT[:, ts(jt, 128)], PTs_v[:, jt, :],
                                 start=(jt == 0), stop=(jt == NT - 1))
            # broadcast r across partitions: bc[c, i] = r[S*512 + i]
            bc = spool.tile([128, 512], f32, tag="sm")
            nc.tensor.matmul(bc[:], ones1[:], rTs[:], start=True, stop=True)
            bcs = work.tile([128, 512], f32, tag="bcs")
            nc.vector.tensor_copy(bcs[:], bc[:])
            tmp = work.tile([128, 512], f32, tag="tmp")
            nc.vector.scalar_tensor_tensor(tmp[:], acc[:], gam[:], bcs[:],
                                           op0=mult, op1=mult)
            osb = work.tile([128, 512], f32, tag="osb")
            nc.vector.tensor_add(osb[:], tmp[:], xb[:, ts(S, 512)])
            nc.sync.dma_start(out_out[:, ts(S, 512)], osb[:])

    nc.compile()
    return nc


def _get_nc():
    if "nc" not in _cache:
        _cache["nc"] = _build()
    return _cache["nc"]


def build_in_maps(inputs):
    x = np.asarray(inputs["x"], dtype=np.float32)
    wq = np.asarray(inputs["wq"], dtype=np.float32)
    wk = np.asarray(inputs["wk"], dtype=np.float32)
    wv = np.asarray(inputs["wv"], dtype=np.float32)
    gamma = np.asarray(inputs["gamma"], dtype=np.float32)

    bf = ml_dtypes.bfloat16
    xf = x.reshape(B, C, N)
    wqt = np.ascontiguousarray(wq.T)
    wkt = np.ascontiguousarray(wk.T)
    wvt = np.ascontiguousarray(wv.T).astype(bf)
    gam = np.full((128, 1), gamma[0], dtype=np.float32)

    in_maps = []
    for b in range(B):
        xb = np.ascontiguousarray(xf[b])
        in_maps.append({
            "x": xb,
            "xb16": xb.astype(bf),
            "wqt": wqt,
            "wkt": wkt,
            "wvt": wvt,
            "gam": gam,
        })
    return in_maps


def kernel(x, wq, wk, wv, gamma, trace=False, tmpdir=None):
    from concourse.bass_utils import run_bass_kernel_spmd

    in_maps = build_in_maps(dict(x=x, wq=wq, wk=wk, wv=wv, gamma=gamma))
    nc = _get_nc()
    res = run_bass_kernel_spmd(nc, in_maps, core_ids=list(range(B)),
                               trace=trace, tmpdir=tmpdir)

    out = np.empty((B, C, N), dtype=np.float32)
    attn = np.empty((B, N, N), dtype=np.float32)
    for b in range(B):
        out[b] = res.results[b]["out"]
        attn[b] = res.results[b]["attn"]

    out = out.reshape(B, C, W, L, H)
    if trace:
        kernel.last_results = res
    return out, attn
